# revision 23
# baseline (speedup 1.0000x reference)
"""Trainium2 Bass kernel for nn_CenterAwarePseudoModule (retrieval_knn).

Reference (per row i of feats, per centroid j):
    f_i   = [feats_i, 1] / ||[feats_i, 1]||
    d2_ij = ||f_i||^2 + ||c_j||^2 - 2 f_i . c_j
    out_i = labelset[argmin_j sqrt(max(d2_ij, 0))]

With q_i = ||feats_i||^2 + 1, h_j = ||c_j||^2 (full row incl. bias col),
G_ij = feats_i . c_j[:D], cb_j = c_j[D]:
    argmin_j d2 = argmax_j (G_ij + cb_j - rh_i * h_j),   rh_i = sqrt(q_i)/2
(positive per-row affine transforms preserve the argmin; validated
empirically against the fp64 oracle: 0 mismatches).

Device strategy (data-parallel over 8 NeuronCores, rows sharded):
  - G via fp8(e4m3) matmuls in DoubleRow perf mode: contraction 256/inst
    at 0.5 cycles/row (2x bf16 PE rate), two 512-col moving chunks per
    group (the ISA 512-moving-element cap; walrus's LDW dedup rejects
    perf-mode LDWEIGHTS, so each matmul self-loads its stationary).
  - bias (cb - rh*h) folded into PSUM by a tiny fp32r matmul first:
    stationary [3,128] = [ones; rh; rh], moving [3,1024] = [cb; -h_hi; -h_lo]
    (h split so fp32r's reduced mantissa on h stays exact).
  - epilogue per 128-row tile: vector.max + max_index straight off PSUM
    [128,1024] (cols >=1000 padded to lose by construction), DMA the TOP-2
    indices out.
  - prologue: bias matmuls for m0-m3 run off the tiny rh/bias DMAs while
    ct/ft stream in; k-major order over m0-m2 tracks the ct prefetch;
    everything is SBUF-resident afterwards (fp8 inputs: 6.3MB/core total).
Host does layout prep (transpose/tiling, e4m3 rounding, norms), an exact
fp64 re-score of each row's device top-2 (so fp8 matmul noise cannot flip
the argmin: a true winner outside the device top-2 needs two independent
>4-sigma fp8 noise events), and the final labelset gather.
"""
import sys

sys.path.insert(0, "/opt/trn_rl_repo")

import numpy as np
import ml_dtypes

N, D, NCENT = 16384, 2048, 1000
NC1024 = 1024            # centroid dim padded to 8 psum chunks of 256
NCORES = 8
R = N // NCORES          # rows per core = 2048
MT = R // 128            # m-tiles per core = 16
KG = D // 256            # DoubleRow contraction groups = 8
HPAD = -2500.0           # pad "-h" value: loses by ~rh*650 for every row

_cache = {}


def _build():
    import concourse.bacc as bacc
    import concourse.tile as tile
    from concourse import mybir

    dt = mybir.dt
    DR = mybir.MatmulPerfMode.DoubleRow

    nc = bacc.Bacc("TRN2", target_bir_lowering=False, debug=False)

    ft = nc.dram_tensor("ft", [MT, 128, KG, 2, 128], dt.float8e4, kind="ExternalInput")
    ct = nc.dram_tensor("ct", [128, KG, 2, NC1024], dt.float8e4, kind="ExternalInput")
    bmv = nc.dram_tensor("bmv", [3, NC1024], dt.float32r, kind="ExternalInput")
    rhd = nc.dram_tensor("rh", [3, MT * 128], dt.float32r, kind="ExternalInput")
    outp = nc.dram_tensor("pred", [MT, 128, 2], dt.uint32, kind="ExternalOutput")

    with tile.TileContext(nc) as tc:
        with (
            tc.tile_pool(name="const", bufs=1) as constp,
            tc.tile_pool(name="epi", bufs=3) as epi,
            tc.tile_pool(name="ps", bufs=4, space="PSUM") as psp,
        ):
            # ---- prologue DMA: tiny bias/rh first (unblocks the PE at t~0),
            # then ct g0 + the first two ft tiles, then the rest of ct, then
            # the rest of ft. Everything is resident for the whole kernel. ----
            bias_sb = constp.tile([3, NC1024], dt.float32r, tag="bias")
            nc.sync.dma_start(bias_sb[:], bmv.ap())
            rh_sb = constp.tile([3, MT * 128], dt.float32r, tag="rh")
            nc.sync.dma_start(rh_sb[:], rhd.ap())
            ct_tiles = [
                constp.tile([128, 2, NC1024], dt.float8e4, tag=f"ct{g}",
                            name=f"ctt{g}")
                for g in range(KG)
            ]
            ft_tiles = [
                constp.tile([128, KG, 2, 128], dt.float8e4, tag=f"ft{m}",
                            name=f"ftt{m}")
                for m in range(MT)
            ]
            nc.sync.dma_start(ct_tiles[0][:], ct.ap()[:, 0])
            for m in range(2):
                nc.sync.dma_start(ft_tiles[m][:], ft.ap()[m])
            for g in range(1, KG):
                nc.sync.dma_start(ct_tiles[g][:], ct.ap()[:, g])
            for m in range(2, MT):
                nc.sync.dma_start(ft_tiles[m][:], ft.ap()[m])

            def bias_mm(ps, m):
                lhs = rh_sb[:, m * 128:(m + 1) * 128]
                nc.tensor.matmul(
                    ps[:, 0:512], lhs, bias_sb[:, 0:512],
                    start=True, stop=False,
                )
                nc.tensor.matmul(
                    ps[:, 512:NC1024], lhs, bias_sb[:, 512:NC1024],
                    start=True, stop=False,
                )

            def g_group(ps, m, g):
                lhs = ft_tiles[m][:, g]
                for ch in range(2):
                    nc.tensor.matmul(
                        ps[:, ch * 512:(ch + 1) * 512],
                        lhs,
                        ct_tiles[g][:, :, ch * 512:(ch + 1) * 512],
                        start=False, stop=(g == KG - 1),
                        perf_mode=DR,
                    )

            def epilogue(ps, m):
                mx = epi.tile([128, 8], dt.float32, tag="mx", name=f"mx{m}")
                nc.vector.max(mx[:], ps[:])
                mi = epi.tile([128, 8], dt.uint32, tag="mi", name=f"mi{m}")
                nc.vector.max_index(mi[:], mx[:], ps[:])
                nc.sync.dma_start(outp.ap()[m], mi[:, 0:2])

            # ---- warm-up: bias matmuls for m0-m3 need only the tiny rh/bias
            # DMAs, so they fill the PE while ct/ft stream in; then m0-m2
            # k-major tracks the ct prefetch. ----
            pss = [
                psp.tile([128, NC1024], dt.float32, tag="ps", name=f"ps{m}")
                for m in range(4)
            ]
            for m in range(4):
                bias_mm(pss[m], m)
            for g in range(KG):
                for m in range(2):
                    g_group(pss[m], m, g)
            for m in range(2):
                epilogue(pss[m], m)

            # -- steady state: m-major (m2/m3 psums are already bias-primed) --
            for m in range(2, MT):
                if m in (2, 3):
                    ps = pss[m]
                else:
                    ps = psp.tile([128, NC1024], dt.float32, tag="ps",
                                  name=f"ps{m}")
                    bias_mm(ps, m)
                for g in range(KG):
                    g_group(ps, m, g)
                epilogue(ps, m)

    nc.compile()
    return nc


def _prep_inputs(feats, initc):
    feats = np.ascontiguousarray(np.asarray(feats, dtype=np.float32))
    initc = np.ascontiguousarray(np.asarray(initc, dtype=np.float32))

    f8 = feats.astype(ml_dtypes.float8_e4m3)
    c8 = initc[:, :D].astype(ml_dtypes.float8_e4m3)

    # ct[p, g, i, j] = c8[j, g*256 + i*128 + p], zero-padded to 1024 centroids
    ctp = np.zeros((128, KG, 2, NC1024), dtype=ml_dtypes.float8_e4m3)
    ctp[:, :, :, :NCENT] = c8.T.reshape(KG, 2, 128, NCENT).transpose(2, 0, 1, 3)

    h = (initc.astype(np.float64) ** 2).sum(axis=1)
    # split h so the PE's reduced-mantissa fp32r input rounding is exact:
    # h_hi has 10 mantissa bits (exact under any >=10-bit PE rounding),
    # h_lo carries the remainder (|h_lo| ~ h * 2^-11, its own rounding moot)
    mant, expo = np.frexp(h)
    h_hi = np.ldexp(np.round(mant * 1024.0) / 1024.0, expo)
    h_lo = (h - h_hi).astype(np.float32)
    bmv = np.zeros((3, NC1024), dtype=np.float32)
    bmv[0, :NCENT] = initc[:, D]
    bmv[1, :NCENT] = -h_hi.astype(np.float32)
    bmv[1, NCENT:] = HPAD
    bmv[2, :NCENT] = -h_lo

    q = (feats.astype(np.float64) ** 2).sum(axis=1) + 1.0
    rh_all = (np.sqrt(q) / 2.0).astype(np.float32)  # [N]

    in_maps = []
    for c in range(NCORES):
        fc = f8[c * R:(c + 1) * R]  # [R, D]
        # ft[m, p, g, i, r] = fc[m*128 + r, (g*2+i)*128 + p]
        X = np.ascontiguousarray(
            fc.reshape(MT, 128, KG, 2, 128).transpose(0, 4, 2, 3, 1)
        )
        rhc = np.empty((3, MT * 128), dtype=np.float32)
        rhc[0] = 1.0
        rhc[1] = rh_all[c * R:(c + 1) * R]
        rhc[2] = rhc[1]
        in_maps.append({"ft": X, "ct": ctp, "bmv": bmv, "rh": rhc})
    return in_maps


def _enable_ldw_opt():
    """walrus dedups back-to-back LDWEIGHTS of the same stationary operand
    when --enable-ldw-opt=true; concourse hardcodes false. NOTE: walrus
    rejects DoubleRow InstLdweights under this flag ("not compatible with
    LDW optimization"), so the fp8 DoubleRow kernel must run without it."""
    import concourse.bass_utils as bu

    if getattr(bu, "_ldw_opt_patched", False):
        return
    orig = bu.run_command

    def patched(argv, **kw):
        argv = [
            "--enable-ldw-opt=true" if a == "--enable-ldw-opt=false" else a
            for a in argv
        ]
        return orig(argv, **kw)

    bu.run_command = patched
    bu._ldw_opt_patched = True


def _refine_top2(feats, initc, cand):
    """Exact (fp64) score comparison of the device's top-2 candidates per
    row; fixes any argmax flip the fp8 matmul noise may have caused. The
    true winner is in the device top-2 with overwhelming probability (a
    displacement needs two independent >4-sigma noise events)."""
    feats = np.asarray(feats, np.float64)
    initc = np.asarray(initc, np.float64)
    h = (initc * initc).sum(axis=1)
    cb = initc[:, D]
    rh = np.sqrt((feats * feats).sum(axis=1) + 1.0) / 2.0
    pred = np.empty(feats.shape[0], dtype=np.int64)
    CH = 2048
    for a in range(0, feats.shape[0], CH):
        b = a + CH
        c2 = initc[cand[a:b], :D]                      # [CH, 2, D]
        g = np.matmul(c2, feats[a:b, :, None])[..., 0]  # [CH, 2]
        s = g + cb[cand[a:b]] - rh[a:b, None] * h[cand[a:b]]
        pick = s[:, 1] > s[:, 0]
        pred[a:b] = np.where(pick, cand[a:b, 1], cand[a:b, 0])
    return pred


def _run(feats, initc, labelset, trace=False):
    from concourse.bass_utils import run_bass_kernel_spmd

    if "nc" not in _cache:
        _cache["nc"] = _build()
    nc = _cache["nc"]

    in_maps = _prep_inputs(feats, initc)
    res = run_bass_kernel_spmd(
        nc, in_maps, core_ids=list(range(NCORES)), trace=trace
    )

    cand = np.concatenate(
        [res.results[c]["pred"].reshape(R, 2) for c in range(NCORES)]
    ).astype(np.int64)
    preds = _refine_top2(feats, initc, cand)
    labelset = np.asarray(labelset)
    out = labelset[preds]
    return out, res


def kernel(feats, initc, labelset):
    out, _ = _run(feats, initc, labelset, trace=False)
    return out


# revision 25
# speedup vs baseline: 1.0049x; 1.0049x over previous
"""Trainium2 Bass kernel for nn_CenterAwarePseudoModule (retrieval_knn).

Reference (per row i of feats, per centroid j):
    f_i   = [feats_i, 1] / ||[feats_i, 1]||
    d2_ij = ||f_i||^2 + ||c_j||^2 - 2 f_i . c_j
    out_i = labelset[argmin_j sqrt(max(d2_ij, 0))]

With q_i = ||feats_i||^2 + 1, h_j = ||c_j||^2 (full row incl. bias col),
G_ij = feats_i . c_j[:D], cb_j = c_j[D]:
    argmin_j d2 = argmax_j (G_ij + cb_j - rh_i * h_j),   rh_i = sqrt(q_i)/2
(positive per-row affine transforms preserve the argmin; validated
empirically against the fp64 oracle: 0 mismatches).

Device strategy (data-parallel over 8 NeuronCores, rows sharded):
  - G via fp8(e4m3) matmuls in DoubleRow perf mode: contraction 256/inst
    at 0.5 cycles/row (2x bf16 PE rate), two 512-col moving chunks per
    group (the ISA 512-moving-element cap; walrus's LDW dedup rejects
    perf-mode LDWEIGHTS, so each matmul self-loads its stationary).
  - bias (cb - rh*h) folded into PSUM by a tiny fp32r matmul first:
    stationary [3,128] = [ones; rh; rh], moving [3,1024] = [cb; -h_hi; -h_lo]
    (h split so fp32r's reduced mantissa on h stays exact).
  - epilogue per 128-row tile: vector.max + max_index straight off PSUM
    [128,1024] (cols >=1000 padded to lose by construction), DMA the TOP-2
    indices out.
  - prologue: bias matmuls for m0-m3 run off the tiny rh/bias DMAs while
    ct/ft stream in; k-major order over m0-m2 tracks the ct prefetch;
    everything is SBUF-resident afterwards (fp8 inputs: 6.3MB/core total).
Host does layout prep (transpose/tiling, e4m3 rounding, norms), an exact
fp64 re-score of each row's device top-2 (so fp8 matmul noise cannot flip
the argmin: a true winner outside the device top-2 needs two independent
>4-sigma fp8 noise events), and the final labelset gather.
"""
import sys

sys.path.insert(0, "/opt/trn_rl_repo")

import numpy as np
import ml_dtypes

N, D, NCENT = 16384, 2048, 1000
NC1024 = 1024            # centroid dim padded to 8 psum chunks of 256
NCORES = 8
R = N // NCORES          # rows per core = 2048
MT = R // 128            # m-tiles per core = 16
KG = D // 256            # DoubleRow contraction groups = 8
HPAD = -2500.0           # pad "-h" value: loses by ~rh*650 for every row

_cache = {}


def _build():
    import concourse.bacc as bacc
    import concourse.tile as tile
    from concourse import mybir

    dt = mybir.dt
    DR = mybir.MatmulPerfMode.DoubleRow

    nc = bacc.Bacc("TRN2", target_bir_lowering=False, debug=False)

    ft = nc.dram_tensor("ft", [MT, 128, KG, 2, 128], dt.float8e4, kind="ExternalInput")
    ct = nc.dram_tensor("ct", [128, KG, 2, NC1024], dt.float8e4, kind="ExternalInput")
    bmv = nc.dram_tensor("bmv", [3, NC1024], dt.float32r, kind="ExternalInput")
    rhd = nc.dram_tensor("rh", [3, MT * 128], dt.float32r, kind="ExternalInput")
    outp = nc.dram_tensor("pred", [MT, 128, 2], dt.uint32, kind="ExternalOutput")

    with tile.TileContext(nc) as tc:
        with (
            tc.tile_pool(name="const", bufs=1) as constp,
            tc.tile_pool(name="epi", bufs=3) as epi,
            tc.tile_pool(name="ps", bufs=4, space="PSUM") as psp,
        ):
            # ---- prologue DMA: tiny bias/rh first (unblocks the PE at t~0),
            # then ct g0 + the first two ft tiles, then the rest of ct, then
            # the rest of ft. Everything is resident for the whole kernel. ----
            bias_sb = constp.tile([3, NC1024], dt.float32r, tag="bias")
            nc.sync.dma_start(bias_sb[:], bmv.ap())
            rh_sb = constp.tile([3, MT * 128], dt.float32r, tag="rh")
            nc.sync.dma_start(rh_sb[:], rhd.ap())
            ct_tiles = [
                constp.tile([128, 2, NC1024], dt.float8e4, tag=f"ct{g}",
                            name=f"ctt{g}")
                for g in range(KG)
            ]
            ft_tiles = [
                constp.tile([128, KG, 2, 128], dt.float8e4, tag=f"ft{m}",
                            name=f"ftt{m}")
                for m in range(MT)
            ]
            # ct streamed in 512-col halves: the ch0 matmul of a group only
            # depends on the first half, so the k-major warm-up starts each
            # group ~0.4us earlier than with whole-tile arrivals.
            def ct_dma(g):
                for ch in range(2):
                    nc.sync.dma_start(
                        ct_tiles[g][:, :, ch * 512:(ch + 1) * 512],
                        ct.ap()[:, g, :, ch * 512:(ch + 1) * 512],
                    )

            ct_dma(0)
            for m in range(3):
                nc.sync.dma_start(ft_tiles[m][:], ft.ap()[m])
            for g in range(1, KG):
                ct_dma(g)
            for m in range(3, MT):
                nc.sync.dma_start(ft_tiles[m][:], ft.ap()[m])

            def bias_mm(ps, m):
                lhs = rh_sb[:, m * 128:(m + 1) * 128]
                nc.tensor.matmul(
                    ps[:, 0:512], lhs, bias_sb[:, 0:512],
                    start=True, stop=False,
                )
                nc.tensor.matmul(
                    ps[:, 512:NC1024], lhs, bias_sb[:, 512:NC1024],
                    start=True, stop=False,
                )

            def g_group(ps, m, g):
                lhs = ft_tiles[m][:, g]
                for ch in range(2):
                    nc.tensor.matmul(
                        ps[:, ch * 512:(ch + 1) * 512],
                        lhs,
                        ct_tiles[g][:, :, ch * 512:(ch + 1) * 512],
                        start=False, stop=(g == KG - 1),
                        perf_mode=DR,
                    )

            def epilogue(ps, m):
                mx = epi.tile([128, 8], dt.float32, tag="mx", name=f"mx{m}")
                nc.vector.max(mx[:], ps[:])
                mi = epi.tile([128, 8], dt.uint32, tag="mi", name=f"mi{m}")
                nc.vector.max_index(mi[:], mx[:], ps[:])
                nc.sync.dma_start(outp.ap()[m], mi[:, 0:2])

            # ---- warm-up: bias matmuls for m0-m3 need only the tiny rh/bias
            # DMAs, so they fill the PE while ct/ft stream in; then m0-m2
            # k-major tracks the ct prefetch. ----
            pss = [
                psp.tile([128, NC1024], dt.float32, tag="ps", name=f"ps{m}")
                for m in range(4)
            ]
            for m in range(4):
                bias_mm(pss[m], m)
            for g in range(KG):
                for m in range(3):
                    g_group(pss[m], m, g)
            for m in range(3):
                epilogue(pss[m], m)

            # ---- steady state: m-major (m3's psum is already bias-primed) --
            for m in range(3, MT):
                if m == 3:
                    ps = pss[3]
                else:
                    ps = psp.tile([128, NC1024], dt.float32, tag="ps",
                                  name=f"ps{m}")
                    bias_mm(ps, m)
                for g in range(KG):
                    g_group(ps, m, g)
                epilogue(ps, m)

    nc.compile()
    return nc


def _prep_inputs(feats, initc):
    feats = np.ascontiguousarray(np.asarray(feats, dtype=np.float32))
    initc = np.ascontiguousarray(np.asarray(initc, dtype=np.float32))

    f8 = feats.astype(ml_dtypes.float8_e4m3)
    c8 = initc[:, :D].astype(ml_dtypes.float8_e4m3)

    # ct[p, g, i, j] = c8[j, g*256 + i*128 + p], zero-padded to 1024 centroids
    ctp = np.zeros((128, KG, 2, NC1024), dtype=ml_dtypes.float8_e4m3)
    ctp[:, :, :, :NCENT] = c8.T.reshape(KG, 2, 128, NCENT).transpose(2, 0, 1, 3)

    h = (initc.astype(np.float64) ** 2).sum(axis=1)
    # split h so the PE's reduced-mantissa fp32r input rounding is exact:
    # h_hi has 10 mantissa bits (exact under any >=10-bit PE rounding),
    # h_lo carries the remainder (|h_lo| ~ h * 2^-11, its own rounding moot)
    mant, expo = np.frexp(h)
    h_hi = np.ldexp(np.round(mant * 1024.0) / 1024.0, expo)
    h_lo = (h - h_hi).astype(np.float32)
    bmv = np.zeros((3, NC1024), dtype=np.float32)
    bmv[0, :NCENT] = initc[:, D]
    bmv[1, :NCENT] = -h_hi.astype(np.float32)
    bmv[1, NCENT:] = HPAD
    bmv[2, :NCENT] = -h_lo

    q = (feats.astype(np.float64) ** 2).sum(axis=1) + 1.0
    rh_all = (np.sqrt(q) / 2.0).astype(np.float32)  # [N]

    in_maps = []
    for c in range(NCORES):
        fc = f8[c * R:(c + 1) * R]  # [R, D]
        # ft[m, p, g, i, r] = fc[m*128 + r, (g*2+i)*128 + p]
        X = np.ascontiguousarray(
            fc.reshape(MT, 128, KG, 2, 128).transpose(0, 4, 2, 3, 1)
        )
        rhc = np.empty((3, MT * 128), dtype=np.float32)
        rhc[0] = 1.0
        rhc[1] = rh_all[c * R:(c + 1) * R]
        rhc[2] = rhc[1]
        in_maps.append({"ft": X, "ct": ctp, "bmv": bmv, "rh": rhc})
    return in_maps


def _enable_ldw_opt():
    """walrus dedups back-to-back LDWEIGHTS of the same stationary operand
    when --enable-ldw-opt=true; concourse hardcodes false. NOTE: walrus
    rejects DoubleRow InstLdweights under this flag ("not compatible with
    LDW optimization"), so the fp8 DoubleRow kernel must run without it."""
    import concourse.bass_utils as bu

    if getattr(bu, "_ldw_opt_patched", False):
        return
    orig = bu.run_command

    def patched(argv, **kw):
        argv = [
            "--enable-ldw-opt=true" if a == "--enable-ldw-opt=false" else a
            for a in argv
        ]
        return orig(argv, **kw)

    bu.run_command = patched
    bu._ldw_opt_patched = True


def _refine_top2(feats, initc, cand):
    """Exact (fp64) score comparison of the device's top-2 candidates per
    row; fixes any argmax flip the fp8 matmul noise may have caused. The
    true winner is in the device top-2 with overwhelming probability (a
    displacement needs two independent >4-sigma noise events)."""
    feats = np.asarray(feats, np.float64)
    initc = np.asarray(initc, np.float64)
    h = (initc * initc).sum(axis=1)
    cb = initc[:, D]
    rh = np.sqrt((feats * feats).sum(axis=1) + 1.0) / 2.0
    pred = np.empty(feats.shape[0], dtype=np.int64)
    CH = 2048
    for a in range(0, feats.shape[0], CH):
        b = a + CH
        c2 = initc[cand[a:b], :D]                      # [CH, 2, D]
        g = np.matmul(c2, feats[a:b, :, None])[..., 0]  # [CH, 2]
        s = g + cb[cand[a:b]] - rh[a:b, None] * h[cand[a:b]]
        pick = s[:, 1] > s[:, 0]
        pred[a:b] = np.where(pick, cand[a:b, 1], cand[a:b, 0])
    return pred


def _run(feats, initc, labelset, trace=False):
    from concourse.bass_utils import run_bass_kernel_spmd

    if "nc" not in _cache:
        _cache["nc"] = _build()
    nc = _cache["nc"]

    in_maps = _prep_inputs(feats, initc)
    res = run_bass_kernel_spmd(
        nc, in_maps, core_ids=list(range(NCORES)), trace=trace
    )

    cand = np.concatenate(
        [res.results[c]["pred"].reshape(R, 2) for c in range(NCORES)]
    ).astype(np.int64)
    preds = _refine_top2(feats, initc, cand)
    labelset = np.asarray(labelset)
    out = labelset[preds]
    return out, res


def kernel(feats, initc, labelset):
    out, _ = _run(feats, initc, labelset, trace=False)
    return out
